# revision 2
# baseline (speedup 1.0000x reference)
"""MLA attention forward kernel for 8 Trainium2 NeuronCores.

Sharding: 2 (batch) x 4 (head-group) grid over 8 cores.
Each core computes, for its batch b and its 8 heads:
  - q_c^T = w_qa^T @ hidden^T   (replicated within the batch group)
  - ckv^T = w_kva^T @ hidden^T  (replicated within the batch group)
  - q^T, k^T (head-dim-major), v (token-major) for its heads
  - causal flash attention (no max subtraction -- scores are O(1) for this
    input distribution), softmax normalization folded into P
  - partial o_proj: out_partial[tok, H] = attn_out^T.T @ w_o_slice
Host sums the 4 partials per batch and takes ckv from one core per batch.

All matmuls run in bf16 with fp32 PSUM accumulation. Activations are kept
feature-major ("transposed") on chip so every matmul has its contraction
dim on partitions without runtime transposes; the only transposes are the
P-tile (softmax prob) transposes, done on the DMA XBAR (bf16 128x128).
"""

import math
import sys

sys.path.insert(0, "/opt/trn_rl_repo")

import numpy as np
import ml_dtypes

BF16 = ml_dtypes.bfloat16

B, S, H = 2, 2048, 4096
NH, HD = 32, 128
QR, KVR = 1536, 512
VD = 128
NCORE = 8
GPB = 4          # head groups per batch (cores per batch)
HPG = NH // GPB  # heads per core = 8
HDG = HPG * HD   # per-core head dim columns = 1024

P = 128
KS_H = H // P     # 32 k-subtiles over hidden dim
KS_QR = QR // P   # 12
KS_KV = KVR // P  # 4
NT = S // P       # 16 token tiles
NG = NT // 4      # 4 q-groups of 512


def build_program(repeat=1):
    import concourse.bass as bass  # noqa: F401
    import concourse.tile as tile
    from concourse import bacc, mybir
    from concourse.masks import make_causal_mask

    f32 = mybir.dt.float32
    bf16 = mybir.dt.bfloat16

    nc = bacc.Bacc(None, target_bir_lowering=False)

    # External I/O (per-core shards, prepared on host)
    hT = nc.dram_tensor("ht", [H, S], bf16, kind="ExternalInput")
    wqa = nc.dram_tensor("wqa", [H, QR], bf16, kind="ExternalInput")
    wkva = nc.dram_tensor("wkva", [H, KVR], bf16, kind="ExternalInput")
    wqb = nc.dram_tensor("wqb", [QR, HDG], bf16, kind="ExternalInput")
    wkb = nc.dram_tensor("wkb", [KVR, HDG], bf16, kind="ExternalInput")
    wvb = nc.dram_tensor("wvb", [KVR, HDG], bf16, kind="ExternalInput")
    wo = nc.dram_tensor("wo", [HDG, H], bf16, kind="ExternalInput")
    outp = nc.dram_tensor("outp", [S, H], f32, kind="ExternalOutput")
    ckv_out = nc.dram_tensor("ckv_out", [S, KVR], f32, kind="ExternalOutput")

    add = mybir.AluOpType.add
    Exp = mybir.ActivationFunctionType.Exp

    with tile.TileContext(nc) as tc:
        with tc.tile_pool(name="dram", bufs=1, space="DRAM") as dram, \
             tc.tile_pool(name="const", bufs=1) as const:
            qcT = dram.tile([QR, S], bf16)    # q_c^T
            ckvT = dram.tile([KVR, S], bf16)  # ckv^T
            qT = dram.tile([HDG, S], bf16)    # per-head stacked q^T
            kT = dram.tile([HDG, S], bf16)
            vtm = dram.tile([S, HDG], bf16)   # v token-major
            aoT = dram.tile([HDG, S], bf16)   # attn out^T

            cmask = const.tile([P, P], f32)
            make_causal_mask(nc, cmask[:], mask_val=-1e9)

            def body():
                # ---- Stage A: q_c^T and ckv^T from hidden^T ----
                with tc.tile_pool(name="a_ht", bufs=1) as a_ht, \
                     tc.tile_pool(name="a_w", bufs=3) as a_w, \
                     tc.tile_pool(name="a_ps", bufs=4, space="PSUM") as a_ps, \
                     tc.tile_pool(name="a_ob", bufs=4) as a_ob:
                    TOKH = S // 2  # 1024 token half
                    hT3 = hT[:].rearrange("(ks p) t -> p ks t", p=P)
                    wqa3 = wqa[:].rearrange("(ks p) m -> p ks m", p=P)
                    wkva3 = wkva[:].rearrange("(ks p) m -> p ks m", p=P)
                    for half in range(2):
                        ht_t = a_ht.tile([P, KS_H, TOKH], bf16, tag="ht")
                        nc.sync.dma_start(
                            out=ht_t[:],
                            in_=hT3[:, :, half * TOKH:(half + 1) * TOKH],
                        )
                        for m in range(KS_QR + KS_KV):
                            is_q = m < KS_QR
                            wsrc = wqa3 if is_q else wkva3
                            mi = m if is_q else m - KS_QR
                            wt = a_w.tile([P, KS_H, P], bf16, tag="wt")
                            nc.sync.dma_start(
                                out=wt[:], in_=wsrc[:, :, mi * P:(mi + 1) * P]
                            )
                            for nsub in range(TOKH // 512):
                                ps = a_ps.tile([P, 512], f32, tag="ps")
                                for ks in range(KS_H):
                                    nc.tensor.matmul(
                                        ps[:],
                                        lhsT=wt[:, ks, :],
                                        rhs=ht_t[:, ks, nsub * 512:(nsub + 1) * 512],
                                        start=(ks == 0),
                                        stop=(ks == KS_H - 1),
                                    )
                                ob = a_ob.tile([P, 512], bf16, tag="ob")
                                nc.any.tensor_copy(out=ob[:], in_=ps[:])
                                dst = qcT if is_q else ckvT
                                tcol = half * TOKH + nsub * 512
                                nc.sync.dma_start(
                                    out=dst[mi * P:(mi + 1) * P, tcol:tcol + 512],
                                    in_=ob[:],
                                )

                # ---- Stage A2: ckv fp32 output (transpose ckv^T) ----
                with tc.tile_pool(name="a2", bufs=4) as a2:
                    for m in range(KS_KV):
                        for t in range(NT):
                            tp = a2.tile([P, P], bf16, tag="tp")
                            nc.sync.dma_start(
                                out=tp[:],
                                in_=ckvT[m * P:(m + 1) * P, t * P:(t + 1) * P],
                                transpose=True,
                            )
                            nc.gpsimd.dma_start(
                                out=ckv_out[t * P:(t + 1) * P, m * P:(m + 1) * P],
                                in_=tp[:],
                            )

                # ---- Stage B: q^T, k^T, v ----
                with tc.tile_pool(name="b_w", bufs=1) as b_w, \
                     tc.tile_pool(name="b_c", bufs=2) as b_c, \
                     tc.tile_pool(name="b_ps", bufs=4, space="PSUM") as b_ps, \
                     tc.tile_pool(name="b_ob", bufs=4) as b_ob:
                    wqb_t = b_w.tile([P, KS_QR, HDG], bf16, tag="wqb")
                    nc.sync.dma_start(
                        out=wqb_t[:], in_=wqb[:].rearrange("(ks p) n -> p ks n", p=P)
                    )
                    wkb_t = b_w.tile([P, KS_KV, HDG], bf16, tag="wkb")
                    nc.sync.dma_start(
                        out=wkb_t[:], in_=wkb[:].rearrange("(ks p) n -> p ks n", p=P)
                    )
                    wvb_t = b_w.tile([P, KS_KV, HDG], bf16, tag="wvb")
                    nc.sync.dma_start(
                        out=wvb_t[:], in_=wvb[:].rearrange("(ks p) n -> p ks n", p=P)
                    )
                    qcT3 = qcT[:].rearrange("(ks p) t -> p ks t", p=P)
                    ckvT3 = ckvT[:].rearrange("(ks p) t -> p ks t", p=P)
                    for c in range(S // 512):
                        qc_t = b_c.tile([P, KS_QR, 512], bf16, tag="qc")
                        nc.sync.dma_start(
                            out=qc_t[:], in_=qcT3[:, :, c * 512:(c + 1) * 512]
                        )
                        ckv_t = b_c.tile([P, KS_KV, 512], bf16, tag="ckv")
                        nc.sync.dma_start(
                            out=ckv_t[:], in_=ckvT3[:, :, c * 512:(c + 1) * 512]
                        )
                        for m in range(HDG // P):  # q^T tiles
                            ps = b_ps.tile([P, 512], f32, tag="ps")
                            for ks in range(KS_QR):
                                nc.tensor.matmul(
                                    ps[:],
                                    lhsT=wqb_t[:, ks, m * P:(m + 1) * P],
                                    rhs=qc_t[:, ks, :],
                                    start=(ks == 0),
                                    stop=(ks == KS_QR - 1),
                                )
                            ob = b_ob.tile([P, 512], bf16, tag="ob")
                            nc.any.tensor_copy(out=ob[:], in_=ps[:])
                            nc.sync.dma_start(
                                out=qT[m * P:(m + 1) * P, c * 512:(c + 1) * 512],
                                in_=ob[:],
                            )
                        for m in range(HDG // P):  # k^T tiles
                            ps = b_ps.tile([P, 512], f32, tag="ps")
                            for ks in range(KS_KV):
                                nc.tensor.matmul(
                                    ps[:],
                                    lhsT=wkb_t[:, ks, m * P:(m + 1) * P],
                                    rhs=ckv_t[:, ks, :],
                                    start=(ks == 0),
                                    stop=(ks == KS_KV - 1),
                                )
                            ob = b_ob.tile([P, 512], bf16, tag="ob")
                            nc.any.tensor_copy(out=ob[:], in_=ps[:])
                            nc.sync.dma_start(
                                out=kT[m * P:(m + 1) * P, c * 512:(c + 1) * 512],
                                in_=ob[:],
                            )
                        for t in range(4):  # v token tiles (untransposed)
                            for nsub in range(HDG // 512):
                                ps = b_ps.tile([P, 512], f32, tag="ps")
                                for ks in range(KS_KV):
                                    nc.tensor.matmul(
                                        ps[:],
                                        lhsT=ckv_t[:, ks, t * P:(t + 1) * P],
                                        rhs=wvb_t[:, ks, nsub * 512:(nsub + 1) * 512],
                                        start=(ks == 0),
                                        stop=(ks == KS_KV - 1),
                                    )
                                ob = b_ob.tile([P, 512], bf16, tag="ob")
                                nc.any.tensor_copy(out=ob[:], in_=ps[:])
                                row = c * 512 + t * P
                                nc.sync.dma_start(
                                    out=vtm[row:row + P,
                                            nsub * 512:(nsub + 1) * 512],
                                    in_=ob[:],
                                )

                # ---- Stage C: causal attention per head ----
                with tc.tile_pool(name="c_h", bufs=2) as c_h, \
                     tc.tile_pool(name="c_prow", bufs=3) as c_prow, \
                     tc.tile_pool(name="c_pt", bufs=2) as c_pt, \
                     tc.tile_pool(name="c_l", bufs=8) as c_l, \
                     tc.tile_pool(name="c_ps", bufs=3, space="PSUM") as c_ps, \
                     tc.tile_pool(name="c_pso", bufs=2, space="PSUM") as c_pso, \
                     tc.tile_pool(name="c_ao", bufs=3) as c_ao:
                    for h in range(HPG):
                        qh = c_h.tile([P, S], bf16, tag="qh")
                        nc.sync.dma_start(out=qh[:], in_=qT[h * P:(h + 1) * P, :])
                        kh = c_h.tile([P, S], bf16, tag="kh")
                        nc.sync.dma_start(out=kh[:], in_=kT[h * P:(h + 1) * P, :])
                        vh = c_h.tile([P, NT, VD], bf16, tag="vh")
                        nc.sync.dma_start(
                            out=vh[:],
                            in_=vtm[:, h * VD:(h + 1) * VD].rearrange(
                                "(jt p) d -> p jt d", p=P
                            ),
                        )
                        for G in range(NG):
                            ptb = c_pt.tile([P, NT, 512], bf16, tag="ptb")
                            for ql in range(4):
                                qt = 4 * G + ql
                                nk = (qt + 1) * P
                                nch = (nk + 511) // 512
                                prow = c_prow.tile([P, S], bf16, tag="prow")
                                lparts = c_l.tile([P, 4], f32, tag="lparts")
                                for kc in range(nch):
                                    w = min(512, nk - kc * 512)
                                    ps = c_ps.tile([P, 512], f32, tag="ps")
                                    nc.tensor.matmul(
                                        ps[:, :w],
                                        lhsT=qh[:, qt * P:(qt + 1) * P],
                                        rhs=kh[:, kc * 512:kc * 512 + w],
                                        start=True,
                                        stop=True,
                                    )
                                    if kc == nch - 1:
                                        nc.vector.tensor_tensor(
                                            ps[:, w - P:w], ps[:, w - P:w],
                                            cmask[:], add,
                                        )
                                    nc.scalar.activation(
                                        prow[:, kc * 512:kc * 512 + w],
                                        ps[:, :w],
                                        Exp,
                                        accum_out=lparts[:, kc:kc + 1],
                                    )
                                lsum = c_l.tile([P, 1], f32, tag="lsum")
                                if nch > 1:
                                    nc.vector.tensor_reduce(
                                        lsum[:], lparts[:, :nch],
                                        axis=mybir.AxisListType.X, op=add,
                                    )
                                else:
                                    nc.vector.tensor_copy(
                                        out=lsum[:], in_=lparts[:, 0:1]
                                    )
                                linv = c_l.tile([P, 1], f32, tag="linv")
                                nc.vector.reciprocal(linv[:], lsum[:])
                                nc.vector.tensor_scalar_mul(
                                    prow[:, :nk], prow[:, :nk], linv[:]
                                )
                                for j in range(qt + 1):
                                    nc.sync.dma_start(
                                        out=ptb[:, j, ql * P:(ql + 1) * P],
                                        in_=prow[:, j * P:(j + 1) * P],
                                        transpose=True,
                                    )
                            pso = c_pso.tile([P, 512], f32, tag="pso")
                            nj = 4 * (G + 1)
                            for j in range(nj):
                                qs = max(0, (j - 4 * G) * P)
                                nc.tensor.matmul(
                                    pso[:, qs:512],
                                    lhsT=vh[:, j, :],
                                    rhs=ptb[:, j, qs:512],
                                    start=(j == 0),
                                    stop=(j == nj - 1),
                                )
                            ao = c_ao.tile([P, 512], bf16, tag="ao")
                            nc.vector.tensor_copy(out=ao[:], in_=pso[:])
                            nc.sync.dma_start(
                                out=aoT[h * P:(h + 1) * P,
                                        G * 512:(G + 1) * 512],
                                in_=ao[:],
                            )

                # ---- Stage D: partial o_proj ----
                with tc.tile_pool(name="d_w", bufs=1) as d_w, \
                     tc.tile_pool(name="d_ps", bufs=4, space="PSUM") as d_ps, \
                     tc.tile_pool(name="d_ob", bufs=4) as d_ob:
                    aot = d_w.tile([P, HDG // P, S], bf16, tag="aot")
                    nc.sync.dma_start(
                        out=aot[:], in_=aoT[:].rearrange("(ks p) t -> p ks t", p=P)
                    )
                    wot = d_w.tile([P, HDG // P, H], bf16, tag="wot")
                    nc.sync.dma_start(
                        out=wot[:], in_=wo[:].rearrange("(ks p) n -> p ks n", p=P)
                    )
                    for t in range(NT):
                        for n in range(H // 512):
                            ps = d_ps.tile([P, 512], f32, tag="ps")
                            for ks in range(HDG // P):
                                nc.tensor.matmul(
                                    ps[:],
                                    lhsT=aot[:, ks, t * P:(t + 1) * P],
                                    rhs=wot[:, ks, n * 512:(n + 1) * 512],
                                    start=(ks == 0),
                                    stop=(ks == HDG // P - 1),
                                )
                            ob = d_ob.tile([P, 512], f32, tag="ob")
                            nc.any.tensor_copy(out=ob[:], in_=ps[:])
                            nc.sync.dma_start(
                                out=outp[t * P:(t + 1) * P,
                                         n * 512:(n + 1) * 512],
                                in_=ob[:],
                            )

            if repeat == 1:
                body()
            else:
                with tc.For_i(0, repeat, 1):
                    body()

    nc.compile()
    return nc


def make_in_maps(hidden_states, w_qa, w_qb, w_kva, w_kb, w_vb, w_o):
    scale = 1.0 / math.sqrt(HD)
    in_maps = []
    wqa_b = np.ascontiguousarray(w_qa.astype(BF16))
    wkva_b = np.ascontiguousarray(w_kva.astype(BF16))
    wqb_s = (w_qb * scale).astype(BF16)
    wkb_b = w_kb.astype(BF16)
    wvb_b = w_vb.astype(BF16)
    wo_b = w_o.astype(BF16)
    hTs = [np.ascontiguousarray(hidden_states[b].T.astype(BF16)) for b in range(B)]
    for core in range(NCORE):
        b, g = core // GPB, core % GPB
        cs = slice(g * HDG, (g + 1) * HDG)
        in_maps.append({
            "ht": hTs[b],
            "wqa": wqa_b,
            "wkva": wkva_b,
            "wqb": np.ascontiguousarray(wqb_s[:, cs]),
            "wkb": np.ascontiguousarray(wkb_b[:, cs]),
            "wvb": np.ascontiguousarray(wvb_b[:, cs]),
            "wo": np.ascontiguousarray(wo_b[cs, :]),
        })
    return in_maps


_NC_CACHE = {}


def run(inputs, repeat=1):
    from concourse.bass_utils import run_bass_kernel_spmd

    if repeat not in _NC_CACHE:
        _NC_CACHE[repeat] = build_program(repeat)
    nc = _NC_CACHE[repeat]
    in_maps = make_in_maps(**{k: np.asarray(v) for k, v in inputs.items()})
    res = run_bass_kernel_spmd(nc, in_maps, core_ids=list(range(NCORE)))
    attn = np.zeros((B, S, H), np.float32)
    for core in range(NCORE):
        attn[core // GPB] += res.results[core]["outp"]
    ckv = np.stack([res.results[b * GPB]["ckv_out"] for b in range(B)])
    return attn, ckv


def kernel(**inputs):
    return run(inputs, repeat=1)


# revision 56
# speedup vs baseline: 58.5200x; 58.5200x over previous
"""MLA attention forward kernel for 8 Trainium2 NeuronCores.

Sharding: 2 (batch) x 4 (head-group) grid over 8 cores.
Per core, for its batch b and its 8 heads:
  - Stage A: the low-rank projections q_c^T / ckv^T (= [w_qa | w_kva]^T @
    hidden^T) are split 4-ways across the batch group by output row-block;
    token-chunked AllGathers ([[0..3],[4..7]]) rebuild the full q_c^T/ckv^T
    on every core while later chunks still compute.
  - Stage B: q^T, k^T (head-dim-major) and v (token-major) for its heads.
  - Stage C: causal flash attention (no max subtraction -- scores are O(1)
    for this input distribution), softmax normalization folded into P.
  - Stage D: partial o_proj out_partial[tok, H] = attn_out^T.T @ w_o_slice.
Host sums the 4 partials per batch and takes ckv from one core per batch.

All matmuls run in bf16 with fp32 PSUM accumulation. Activations are kept
feature-major ("transposed") on chip so every matmul has its contraction
dim on partitions; the only transposes are the P-tile (softmax prob)
transposes, done on the DMA XBAR one whole causal row per instruction, and
the ckv output transpose.

Intermediates live in DRAM in partition-tiled layout [128, nblk, S] so each
load/store is a single large-descriptor DMA (HWDGE issue overhead ~630ns
per instruction makes many small DMAs the dominant cost otherwise).
"""

import math
import sys

sys.path.insert(0, "/opt/trn_rl_repo")

import numpy as np
import ml_dtypes

BF16 = ml_dtypes.bfloat16

B, S, H = 2, 2048, 4096
NH, HD = 32, 128
QR, KVR = 1536, 512
VD = 128
NCORE = 8
GPB = 4          # head groups per batch (cores per batch)
HPG = NH // GPB  # heads per core = 8
HDG = HPG * HD   # per-core head dim columns = 1024

P = 128
KS_H = H // P     # 32 k-subtiles over hidden dim
KS_QR = QR // P   # 12
KS_KV = KVR // P  # 4
NT = S // P       # 16 token tiles
NG = NT // 4      # 4 q-groups of 512
NM_A = KS_QR + KS_KV  # 16 stage-A output row blocks


def build_program(repeat=1):
    import concourse.bass as bass  # noqa: F401
    import concourse.tile as tile
    from concourse import bacc, mybir

    f32 = mybir.dt.float32
    bf16 = mybir.dt.bfloat16

    nc = bacc.Bacc(None, target_bir_lowering=False, num_devices=NCORE)

    # External I/O (per-core shards, prepared on host).
    # waqt: this core's 4 stage-A weight blocks (of w_qa|w_kva columns),
    # host-pretiled to [ml, p, ks, 128] so the load is one DMA with an
    # 8KB contiguous run per partition.
    hT = nc.dram_tensor("ht", [H, S], bf16, kind="ExternalInput")
    cmask_in = nc.dram_tensor("cmask", [P, P], f32, kind="ExternalInput")
    waqt = nc.dram_tensor("waqt", [NM_A // GPB, P, KS_H, P], bf16,
                          kind="ExternalInput")
    wqb = nc.dram_tensor("wqb", [QR, HDG], bf16, kind="ExternalInput")
    wkb = nc.dram_tensor("wkb", [KVR, HDG], bf16, kind="ExternalInput")
    wvb = nc.dram_tensor("wvb", [KVR, HDG], bf16, kind="ExternalInput")
    wo = nc.dram_tensor("wo", [HDG, H], bf16, kind="ExternalInput")
    outp = nc.dram_tensor("outp", [S, H], f32, kind="ExternalOutput")
    ckv_out = nc.dram_tensor("ckv_out", [S, KVR], f32, kind="ExternalOutput")

    add = mybir.AluOpType.add
    Exp = mybir.ActivationFunctionType.Exp

    with tile.TileContext(nc) as tc:
        with tc.tile_pool(name="dram", bufs=1, space="DRAM") as dram, \
             tc.tile_pool(name="const", bufs=1) as const:
            # partition-tiled intermediates: [128, row-block, token]
            qT = dram.tile([P, HPG, S], bf16)
            kT = dram.tile([P, HPG, S], bf16)
            vt = dram.tile([P, NT, HDG], bf16)   # v[tok, d] tiled over tokens

            cmask = const.tile([P, P], f32)
            nc.sync.dma_start(out=cmask[:], in_=cmask_in[:])

            # Shared DRAM for the stage-A allgather: each core computes its
            # 4 row-blocks (of 16) for every 512-token chunk; allgather c
            # delivers all 16 row-blocks for chunk c.
            NMG = NM_A // GPB  # 4 row-blocks per core
            TOKC = 1024  # tokens per allgather chunk
            NCC = S // TOKC
            with tc.tile_pool(name="cc", bufs=1, space="DRAM") as cc_pool:
                loc_c = [cc_pool.tile([P, NMG, TOKC], bf16, name=f"loc{c}")
                         for c in range(NCC)]
                gath_c = [cc_pool.tile([GPB, P, NMG, TOKC], bf16,
                                       name=f"gath{c}")
                          for c in range(NCC)]

            groups = [list(range(GPB)), list(range(GPB, NCORE))]

            def stage_ab():
                with tc.tile_pool(name="b_w", bufs=1) as b_w:
                    # ---- Stage A (this core's row-blocks, all tokens) ----
                    with tc.tile_pool(name="a_ht", bufs=2) as a_ht, \
                         tc.tile_pool(name="a_w", bufs=1) as a_w, \
                         tc.tile_pool(name="a_wide", bufs=3) as a_wide, \
                         tc.tile_pool(name="a_ps", bufs=4, space="PSUM") as a_ps:
                        hT3 = hT[:].rearrange("(ks p) t -> p ks t", p=P)
                        wa_t = a_w.tile([P, NMG, KS_H, P], bf16, tag="wa")
                        nc.scalar.dma_start(
                            out=wa_t[:],
                            in_=waqt[:].rearrange("ml p ks mc -> p ml ks mc"),
                        )
                        for cc in range(NCC):
                            for sub in range(TOKC // 512):
                                t0 = cc * TOKC + sub * 512
                                ht_t = a_ht.tile([P, KS_H, 512], bf16, tag="ht")
                                for kq in range(4):  # chunked for early start
                                    nc.sync.dma_start(
                                        out=ht_t[:, kq * 8:(kq + 1) * 8, :],
                                        in_=hT3[:, kq * 8:(kq + 1) * 8,
                                                t0:t0 + 512],
                                    )
                                wide = a_wide.tile([P, NMG, 512], bf16,
                                                   tag="wide")
                                for ml in range(NMG):
                                    ps = a_ps.tile([P, 512], f32, tag="ps")
                                    for ks in range(KS_H):
                                        nc.tensor.matmul(
                                            ps[:],
                                            lhsT=wa_t[:, ml, ks, :],
                                            rhs=ht_t[:, ks, :],
                                            start=(ks == 0),
                                            stop=(ks == KS_H - 1),
                                        )
                                    nc.any.tensor_copy(
                                        out=wide[:, ml, :], in_=ps[:]
                                    )
                                nc.scalar.dma_start(
                                    out=loc_c[cc][:, :, sub * 512:
                                                   (sub + 1) * 512],
                                    in_=wide[:],
                                )
                            nc.gpsimd.collective_compute(
                                "AllGather",
                                mybir.AluOpType.bypass,
                                replica_groups=groups,
                                ins=[loc_c[cc][:]],
                                outs=[gath_c[cc][:]],
                            )

                    # stage-B weights: issued after stage A's DMAs so the
                    # allgather inputs aren't stuck behind them in the queues
                    wqb_t = b_w.tile([P, KS_QR, HDG], bf16, tag="wqb")
                    nc.scalar.dma_start(
                        out=wqb_t[:],
                        in_=wqb[:].rearrange("(ks p) n -> p ks n", p=P),
                    )
                    wkb_t = b_w.tile([P, KS_KV, HDG], bf16, tag="wkb")
                    nc.scalar.dma_start(
                        out=wkb_t[:],
                        in_=wkb[:].rearrange("(ks p) n -> p ks n", p=P),
                    )
                    wvb_t = b_w.tile([P, KS_KV, HDG], bf16, tag="wvb")
                    nc.scalar.dma_start(
                        out=wvb_t[:],
                        in_=wvb[:].rearrange("(ks p) n -> p ks n", p=P),
                    )

                    # stage-B weights: issued after stage A's DMAs so the
                    # allgather inputs aren't stuck behind them in the queues
                    wqb_t = b_w.tile([P, KS_QR, HDG], bf16, tag="wqb")
                    nc.scalar.dma_start(
                        out=wqb_t[:],
                        in_=wqb[:].rearrange("(ks p) n -> p ks n", p=P),
                    )
                    wkb_t = b_w.tile([P, KS_KV, HDG], bf16, tag="wkb")
                    nc.scalar.dma_start(
                        out=wkb_t[:],
                        in_=wkb[:].rearrange("(ks p) n -> p ks n", p=P),
                    )
                    wvb_t = b_w.tile([P, KS_KV, HDG], bf16, tag="wvb")
                    nc.scalar.dma_start(
                        out=wvb_t[:],
                        in_=wvb[:].rearrange("(ks p) n -> p ks n", p=P),
                    )

                    # ---- Stage A2: fp32 [tok, rank] ckv output ----
                    # ckv row-blocks are global m=12..15 = slot 3, ml 0..3
                    with tc.tile_pool(name="a2", bufs=2) as a2:
                        ck3 = ckv_out[:].rearrange("(t p) r -> p t r", p=P)
                        NTC = TOKC // P
                        for cc in range(NCC):
                            for ml in range(KS_KV):
                                tp = a2.tile([P, NTC, P], bf16, tag="tp")
                                nc.sync.dma_start(
                                    out=tp[:], in_=gath_c[cc][GPB - 1, :, ml, :],
                                    transpose=True,
                                )
                                tpf = a2.tile([P, NTC, P], f32, tag="tpf")
                                nc.vector.tensor_copy(out=tpf[:], in_=tp[:])
                                nc.scalar.dma_start(
                                    out=ck3[:, cc * NTC:(cc + 1) * NTC,
                                            ml * P:(ml + 1) * P],
                                    in_=tpf[:],
                                )

                    # ---- Stage B ----
                    with tc.tile_pool(name="b_c", bufs=2) as b_c, \
                         tc.tile_pool(name="b_wide", bufs=3) as b_wide, \
                         tc.tile_pool(name="b_ps", bufs=6, space="PSUM") as b_ps:
                        for c in range(S // 512):
                            cs = slice(c * 512, (c + 1) * 512)
                            cidx = c * 512 // TOKC
                            toff = c * 512 % TOKC
                            tsl = slice(toff, toff + 512)
                            qc_t = b_c.tile([P, KS_QR, 512], bf16, tag="qc")
                            for g in range(GPB - 1):
                                nc.sync.dma_start(
                                    out=qc_t[:, g * NMG:(g + 1) * NMG, :],
                                    in_=gath_c[cidx][g, :, :, tsl],
                                )
                            ckv_t = b_c.tile([P, KS_KV, 512], bf16, tag="ckv")
                            nc.sync.dma_start(
                                out=ckv_t[:], in_=gath_c[cidx][GPB - 1, :, :, tsl]
                            )
                            qw = b_wide.tile([P, HPG, 512], bf16, tag="qw")
                            kw = b_wide.tile([P, HPG, 512], bf16, tag="kw")
                            for m in range(HPG):  # q^T tiles
                                ps = b_ps.tile([P, 512], f32, tag="ps")
                                for ks in range(KS_QR):
                                    nc.tensor.matmul(
                                        ps[:],
                                        lhsT=wqb_t[:, ks, m * P:(m + 1) * P],
                                        rhs=qc_t[:, ks, :],
                                        start=(ks == 0),
                                        stop=(ks == KS_QR - 1),
                                    )
                                nc.any.tensor_copy(out=qw[:, m, :], in_=ps[:])
                            nc.scalar.dma_start(out=qT[:, :, cs], in_=qw[:])
                            for m in range(HPG):  # k^T tiles
                                ps = b_ps.tile([P, 512], f32, tag="ps")
                                for ks in range(KS_KV):
                                    nc.tensor.matmul(
                                        ps[:],
                                        lhsT=wkb_t[:, ks, m * P:(m + 1) * P],
                                        rhs=ckv_t[:, ks, :],
                                        start=(ks == 0),
                                        stop=(ks == KS_KV - 1),
                                    )
                                nc.any.tensor_copy(out=kw[:, m, :], in_=ps[:])
                            nc.scalar.dma_start(out=kT[:, :, cs], in_=kw[:])
                            for t in range(4):  # v token tiles (untransposed)
                                vw = b_wide.tile([P, HDG], bf16, tag="vw")
                                for nsub in range(HDG // 512):
                                    ps = b_ps.tile([P, 512], f32, tag="ps")
                                    for ks in range(KS_KV):
                                        nc.tensor.matmul(
                                            ps[:],
                                            lhsT=ckv_t[:, ks, t * P:(t + 1) * P],
                                            rhs=wvb_t[:, ks,
                                                      nsub * 512:(nsub + 1) * 512],
                                            start=(ks == 0),
                                            stop=(ks == KS_KV - 1),
                                        )
                                    nc.any.tensor_copy(
                                        out=vw[:, nsub * 512:(nsub + 1) * 512],
                                        in_=ps[:],
                                    )
                                nc.scalar.dma_start(
                                    out=vt[:, c * 4 + t, :], in_=vw[:]
                                )


            def stage_cd():
                with tc.tile_pool(name="cd_w", bufs=1) as cd_w:
                    # prefetch o_proj weights during attention; attn output
                    # stays resident in SBUF between stages C and D.
                    wot = cd_w.tile([P, HPG, H], bf16, tag="wot")
                    nc.scalar.dma_start(
                        out=wot[:], in_=wo[:].rearrange("(ks p) n -> p ks n", p=P)
                    )
                    aot = cd_w.tile([P, HPG, S], bf16, tag="aot")
                    stage_c(aot, wot)
                    stage_d(aot, wot)

            def stage_c(aot, wot):
                with tc.tile_pool(name="c_h", bufs=2) as c_h, \
                     tc.tile_pool(name="c_prow", bufs=4) as c_prow, \
                     tc.tile_pool(name="c_pt", bufs=3) as c_pt, \
                     tc.tile_pool(name="c_l", bufs=8) as c_l, \
                     tc.tile_pool(name="c_ps", bufs=4, space="PSUM") as c_ps, \
                     tc.tile_pool(name="c_pso", bufs=2, space="PSUM") as c_pso:
                    def emit_pv(h, G, ptb, vh):
                        pso = c_pso.tile([P, 512], f32, tag="pso", name="pso")
                        nj = 4 * (G + 1)
                        for j in range(nj):
                            qs = max(0, (j - 4 * G) * P)
                            nc.tensor.matmul(
                                pso[:, qs:512],
                                lhsT=vh[:, j, :],
                                rhs=ptb[:, j, qs:512],
                                start=(j == 0),
                                stop=(j == nj - 1),
                            )
                        nc.vector.tensor_copy(
                            out=aot[:, h, G * 512:(G + 1) * 512], in_=pso[:]
                        )

                    pending = []  # defer PV so PE never starves
                    for h in range(HPG):
                        qh = c_h.tile([P, S], bf16, tag="qh")
                        nc.scalar.dma_start(out=qh[:], in_=qT[:, h, :])
                        kh = c_h.tile([P, S], bf16, tag="kh")
                        nc.scalar.dma_start(out=kh[:], in_=kT[:, h, :])
                        vh = c_h.tile([P, NT, VD], bf16, tag="vh")
                        nc.scalar.dma_start(
                            out=vh[:], in_=vt[:, :, h * VD:(h + 1) * VD]
                        )

                        for G in range(NG):
                            ptb = c_pt.tile([P, NT, 512], bf16, tag="ptb")
                            for ql in range(4):
                                qt = 4 * G + ql
                                nk = (qt + 1) * P
                                nch = (nk + 511) // 512
                                prow = c_prow.tile([P, S], bf16, tag="prow")
                                lparts = c_l.tile([P, 4], f32, tag="lparts")
                                for kc in range(nch):
                                    w = min(512, nk - kc * 512)
                                    ps = c_ps.tile([P, 512], f32, tag="ps")
                                    nc.tensor.matmul(
                                        ps[:, :w],
                                        lhsT=qh[:, qt * P:(qt + 1) * P],
                                        rhs=kh[:, kc * 512:kc * 512 + w],
                                        start=True,
                                        stop=True,
                                    )
                                    if kc == nch - 1:
                                        nc.vector.tensor_tensor(
                                            ps[:, w - P:w], ps[:, w - P:w],
                                            cmask[:], add,
                                        )
                                    nc.scalar.activation(
                                        prow[:, kc * 512:kc * 512 + w],
                                        ps[:, :w],
                                        Exp,
                                        accum_out=lparts[:, kc:kc + 1],
                                    )
                                if nch > 1:
                                    lsum = c_l.tile([P, 1], f32, tag="lsum")
                                    nc.vector.tensor_reduce(
                                        lsum[:], lparts[:, :nch],
                                        axis=mybir.AxisListType.X, op=add,
                                    )
                                else:
                                    lsum = lparts[:, 0:1]
                                linv = c_l.tile([P, 1], f32, tag="linv")
                                nc.vector.reciprocal(linv[:], lsum[:])
                                nc.vector.tensor_scalar_mul(
                                    prow[:, :nk], prow[:, :nk], linv[:]
                                )
                                # one XBAR transpose for the whole causal row:
                                # ptb[p, j, ql*128+q] = prow[q, j*128+p]
                                nc.sync.dma_start(
                                    out=ptb[:, :qt + 1, ql * P:(ql + 1) * P],
                                    in_=prow[:, :nk],
                                    transpose=True,
                                )
                            pending.append((h, G, ptb, vh))
                            if len(pending) > 1:
                                emit_pv(*pending.pop(0))
                    for args in pending:
                        emit_pv(*args)

            def stage_d(aot, wot):
                with tc.tile_pool(name="d_wide", bufs=2) as d_wide, \
                     tc.tile_pool(name="d_ps", bufs=6, space="PSUM") as d_ps:
                    for t in range(NT):
                        ow = d_wide.tile([P, H], f32, tag="ow")
                        for n in range(H // 512):
                            ps = d_ps.tile([P, 512], f32, tag="ps")
                            for ks in range(HPG):
                                nc.tensor.matmul(
                                    ps[:],
                                    lhsT=aot[:, ks, t * P:(t + 1) * P],
                                    rhs=wot[:, ks, n * 512:(n + 1) * 512],
                                    start=(ks == 0),
                                    stop=(ks == HPG - 1),
                                )
                            nc.any.tensor_copy(
                                out=ow[:, n * 512:(n + 1) * 512], in_=ps[:]
                            )
                        nc.scalar.dma_start(
                            out=outp[t * P:(t + 1) * P, :], in_=ow[:]
                        )

            def body():
                stage_ab()
                stage_cd()

            for _ in range(repeat):
                body()

    nc.compile()
    return nc


def make_in_maps(hidden_states, w_qa, w_qb, w_kva, w_kb, w_vb, w_o):
    scale = 1.0 / math.sqrt(HD)
    in_maps = []
    # stage-A weights: concat columns [w_qa | w_kva] -> [H, QR+KVR], then
    # pretile to [m, p, ks, 128] where row = ks*128 + p, col = m*128 + mc.
    wa = np.concatenate(
        [w_qa.astype(BF16), w_kva.astype(BF16)], axis=1
    )  # [H, QR+KVR]
    waqt = np.ascontiguousarray(
        wa.reshape(KS_H, P, NM_A, P).transpose(2, 1, 0, 3)
    )  # [m, p, ks, mc]
    wqb_s = (w_qb * scale).astype(BF16)
    wkb_b = w_kb.astype(BF16)
    wvb_b = w_vb.astype(BF16)
    wo_b = w_o.astype(BF16)
    hTs = [np.ascontiguousarray(hidden_states[b].T.astype(BF16)) for b in range(B)]
    cmask = np.where(
        np.tril(np.ones((P, P), dtype=bool)), 0.0, -1e9
    ).astype(np.float32)
    nmg = NM_A // GPB
    for core in range(NCORE):
        b, g = core // GPB, core % GPB
        cs = slice(g * HDG, (g + 1) * HDG)
        in_maps.append({
            "ht": hTs[b],
            "cmask": cmask,
            "waqt": np.ascontiguousarray(waqt[g * nmg:(g + 1) * nmg]),
            "wqb": np.ascontiguousarray(wqb_s[:, cs]),
            "wkb": np.ascontiguousarray(wkb_b[:, cs]),
            "wvb": np.ascontiguousarray(wvb_b[:, cs]),
            "wo": np.ascontiguousarray(wo_b[cs, :]),
        })
    return in_maps


_NC_CACHE = {}


def run(inputs, repeat=1):
    from concourse.bass_utils import run_bass_kernel_spmd

    if repeat not in _NC_CACHE:
        _NC_CACHE[repeat] = build_program(repeat)
    nc = _NC_CACHE[repeat]
    in_maps = make_in_maps(**{k: np.asarray(v) for k, v in inputs.items()})
    res = run_bass_kernel_spmd(nc, in_maps, core_ids=list(range(NCORE)))
    attn = np.zeros((B, S, H), np.float32)
    for core in range(NCORE):
        attn[core // GPB] += res.results[core]["outp"]
    ckv = np.stack([res.results[b * GPB]["ckv_out"] for b in range(B)])
    return attn, ckv


def kernel(**inputs):
    return run(inputs, repeat=1)
